# revision 2
# baseline (speedup 1.0000x reference)
"""DSA indexer kernel for Trainium2 (8 NeuronCores, SPMD).

Reference computation (nn_DSAIndexer):
    q = rope(q_resid @ W_qb)                          [S, H, D]
    k = rope(LN(hidden @ W_k))                        [T, D]
    w = (hidden @ W_w) * H^-0.5                       [S, H]
    index_scores = einsum('sht,sh->st', relu(q.k^T) * D^-0.5, w)
    out = full descending argsort of index_scores (top_k with k == T)

Sharding: sequence-parallel over S. Each core owns S_LOC = 256 queries:
it computes full K (all T) redundantly, its own q/w slice, its slice of
index_scores, and a full descending argsort (bitonic network, exact fp32
keys + uint16 index payload) of its 256 rows.

|w|*D^-0.5 is folded into q before scoring (w*relu(x) = sign(w)*relu(|w|x));
the sign and head-sum are applied by a second matmul with block-sparse
sign stationaries, accumulating a whole 32-query band per PSUM tile.

Self-contained: hardcodes all shapes; builds and compiles on first call.
"""

import numpy as np

import concourse.bass as bass
import concourse.mybir as mybir
import concourse.tile as tile
from concourse.masks import make_identity

S, T, H, D, R = 2048, 2048, 32, 128, 64
HID, QL = 4096, 1536
NCORES = 8
SL = S // NCORES            # 256 queries per core
NST = SL // 128             # s-tiles per core (2)
NKC = HID // 128            # hidden contraction chunks (32)
NQC = QL // 128             # qlora contraction chunks (12)
QSCALE = float(H) ** -0.5 * float(D) ** -0.5
EPS = 1e-6
HR = R // 2                 # rope half (32)

f32 = mybir.dt.float32
u8 = mybir.dt.uint8
u16 = mybir.dt.uint16
i16 = mybir.dt.int16
i32 = mybir.dt.int32
AX = mybir.AxisListType
ALU = mybir.AluOpType
ACTF = mybir.ActivationFunctionType

# The walrus build in this container rejects instructions carrying more
# than one lowered sync wait ("Too many sync wait commands"); Tile
# freely emits several. After lowering, move excess waits onto
# same-engine no-ops inserted right before the over-subscribed
# instruction (engine program order preserves the wait semantics).
MAX_WAITS = 1


def _split_excess_waits(nc):
    n_split = 0
    for bb in nc.main_func.blocks:
        insts = bb.instructions
        out = []
        for inst in insts:
            si = inst.sync_info
            if si is not None and si.on_wait and len(si.on_wait) > MAX_WAITS:
                waits = list(si.on_wait)
                excess, keep = waits[:-MAX_WAITS], waits[-MAX_WAITS:]
                for i in range(0, len(excess), MAX_WAITS):
                    out.append(mybir.InstNoOp(
                        name=nc.get_next_instruction_name(),
                        engine=inst.engine,
                        ins=[], outs=[],
                        sync_info=mybir.SyncInfo(
                            on_wait=excess[i:i + MAX_WAITS], on_update=[]),
                    ))
                    n_split += 1
                inst.sync_info = mybir.SyncInfo(
                    on_wait=keep, on_update=list(si.on_update or []))
            out.append(inst)
        if n_split:
            insts[:] = out
    return n_split


def _hview(ap, nh, nd):
    return ap.rearrange("p (h d) -> p h d", h=nh)


def _bch(ap, nh, nd):
    """[128, nd] -> [128, nh, nd] broadcast over heads (step 0)."""
    return ap.unsqueeze(1).to_broadcast([ap.shape[0], nh, nd])


def _build(debug_scores=False):
    nc = bass.Bass()

    hidT = nc.dram_tensor("hidT", [HID, S], f32, kind="ExternalInput")
    hownT = nc.dram_tensor("hownT", [HID, SL], f32, kind="ExternalInput")
    qrT = nc.dram_tensor("qrT", [QL, SL], f32, kind="ExternalInput")
    cosT = nc.dram_tensor("cosT", [R, S], f32, kind="ExternalInput")
    sinT = nc.dram_tensor("sinT", [R, S], f32, kind="ExternalInput")
    cosq = nc.dram_tensor("cosq", [SL, R], f32, kind="ExternalInput")
    sinq = nc.dram_tensor("sinq", [SL, R], f32, kind="ExternalInput")
    w_qb = nc.dram_tensor("w_qb", [QL, H * D], f32, kind="ExternalInput")
    w_k = nc.dram_tensor("w_k", [HID, D], f32, kind="ExternalInput")
    w_w = nc.dram_tensor("w_w", [HID, H], f32, kind="ExternalInput")
    gammaC = nc.dram_tensor("gammaC", [D, 1], f32, kind="ExternalInput")
    betaC = nc.dram_tensor("betaC", [D, 1], f32, kind="ExternalInput")
    idx_out = nc.dram_tensor("idx_out", [SL, T], i32, kind="ExternalOutput")
    if debug_scores:
        sc_out = nc.dram_tensor("sc_out", [SL, T], f32, kind="ExternalOutput")

    with tile.TileContext(nc) as tc:
      with tc.tile_pool(name="persist", bufs=1) as persist, \
           tc.tile_pool(name="qkeep", bufs=1) as qk_, \
           tc.tile_pool(name="qtmp", bufs=1) as qtmp, \
           tc.tile_pool(name="qphase", bufs=3) as qp:
        kT = persist.tile([128, T], f32, tag="kT")          # K^T [d, t]
        qT = persist.tile([128, H * SL], f32, tag="qT")     # Q^T [d, (s,h)]
        sgnstat = persist.tile([128, 32 * (SL // 4)], f32, tag="sgnstat")
        is_sb = [persist.tile([128, T], f32, tag=f"is{st}", name=f"is{st}")
                 for st in range(NST)]
        ident = persist.tile([128, 128], f32, tag="ident")
        make_identity(nc, ident[:])
        gb = persist.tile([128, 1], f32, tag="gb")
        nc.sync.dma_start(gb[:], gammaC[:])
        bb_ = persist.tile([128, 1], f32, tag="bb")
        nc.sync.dma_start(bb_[:], betaC[:])
        abswt = [persist.tile([128, H], f32, tag=f"abswt{i}",
                              name=f"abswt{i}") for i in range(NST)]
        qr_t = qk_.tile([128, NQC * SL], f32, tag="qr")
        nc.sync.dma_start(
            qr_t[:].rearrange("p (c s) -> p c s", c=NQC),
            qrT[:].rearrange("(c p) s -> p c s", p=128))
        qsc = [qk_.tile([128, H * D], f32, tag=f"qsc{i}", name=f"qsc{i}")
               for i in range(NST)]

        kwq_psum = tc.tile_pool(name="qpsum", bufs=1, space="PSUM")
        qps = kwq_psum.__enter__()
        kwq_tps = tc.tile_pool(name="qtps", bufs=2, space="PSUM")
        qtps = kwq_tps.__enter__()

        # ------- phase K: k-proj direct in K^T layout -----------------
        # kT[d, t] = rope(LN(hidden @ W_k))^T. LayerNorm stats are per-t
        # (partition axis here), computed via ones-matmuls and broadcast
        # back with a rank-1 matmul; rope's rotate_half is a constant
        # 64x64 rotation matrix applied on the PE.
        with tc.tile_pool(name="kw", bufs=1) as kw, \
             tc.tile_pool(name="kphase", bufs=3) as kp, \
             tc.tile_pool(name="kstat", bufs=1) as kstat:
            epst = kw.tile([128, 1], f32, tag="epst")
            nc.vector.memset(epst[:], EPS)
            ones_d = kw.tile([128, 1], f32, tag="ones_d")
            nc.vector.memset(ones_d[:], 1.0)
            ones_p = kw.tile([1, 128], f32, tag="ones_p")
            nc.vector.memset(ones_p[:], 1.0)
            prot = kw.tile([R, R], f32, tag="prot")
            nc.gpsimd.memset(prot[:], 0.0)
            # prot[32+j, j] = -1 ; prot[j, 32+j] = +1
            nc.gpsimd.affine_select(
                out=prot[:], in_=prot[:], compare_op=ALU.not_equal,
                fill=-1.0, base=-HR, pattern=[[-1, R]], channel_multiplier=1)
            nc.gpsimd.affine_select(
                out=prot[:], in_=prot[:], compare_op=ALU.not_equal,
                fill=1.0, base=HR, pattern=[[-1, R]], channel_multiplier=1)

            kt_raw = kstat.tile([128, S], f32, tag="kt_raw")
            with tc.tile_pool(name="kpsA", bufs=1, space="PSUM") as kpsA:
                pkt = [kpsA.tile([128, 512], f32, tag=f"pkt{i}",
                                 name=f"pkt{i}") for i in range(4)]
                for c in range(NKC):
                    hcc = kp.tile([128, S], f32, tag="hcc")
                    nc.sync.dma_start(hcc[:], hidT[c * 128:(c + 1) * 128, :])
                    wkc = kp.tile([128, D], f32, tag="wkc")
                    nc.sync.dma_start(wkc[:], w_k[c * 128:(c + 1) * 128, :])
                    for tch in range(4):
                        nc.tensor.matmul(
                            pkt[tch][:], wkc[:],
                            hcc[:, tch * 512:(tch + 1) * 512],
                            start=(c == 0), stop=(c == NKC - 1))
                for tch in range(4):
                    nc.scalar.copy(kt_raw[:, tch * 512:(tch + 1) * 512],
                                   pkt[tch][:])
            cosT_sb = kp.tile([R, S], f32, tag="hcc")
            nc.sync.dma_start(cosT_sb[:], cosT[:])
            sinT_sb = kp.tile([R, S], f32, tag="hcc")
            nc.sync.dma_start(sinT_sb[:], sinT[:])
            with tc.tile_pool(name="kpsB", bufs=1, space="PSUM") as kpsB:
                # mean over d (partitions): mu[1, t] = ones_d^T @ kt_raw / D
                pmu = kpsB.tile([1, S], f32, tag="big")
                for tch in range(4):
                    nc.tensor.matmul(
                        pmu[:, tch * 512:(tch + 1) * 512], ones_d[:],
                        kt_raw[:, tch * 512:(tch + 1) * 512],
                        start=True, stop=True)
                mu_sb = kstat.tile([1, S], f32, tag="mu_sb")
                nc.scalar.mul(mu_sb[:], pmu[:], 1.0 / D)
                pmub = kpsB.tile([128, S], f32, tag="big")
                for tch in range(4):
                    nc.tensor.matmul(
                        pmub[:, tch * 512:(tch + 1) * 512], ones_p[:],
                        mu_sb[:, tch * 512:(tch + 1) * 512],
                        start=True, stop=True)
                xc = kstat.tile([128, S], f32, tag="xc")
                nc.vector.tensor_sub(xc[:], kt_raw[:], pmub[:])
                sq = kstat.tile([128, S], f32, tag="kt_raw")
                nc.scalar.activation(sq[:], xc[:], ACTF.Square)
                pssq = kpsB.tile([1, S], f32, tag="big")
                for tch in range(4):
                    nc.tensor.matmul(
                        pssq[:, tch * 512:(tch + 1) * 512], ones_d[:],
                        sq[:, tch * 512:(tch + 1) * 512],
                        start=True, stop=True)
                stdr = kstat.tile([1, S], f32, tag="stdr")
                nc.scalar.activation(stdr[:], pssq[:], ACTF.Sqrt,
                                     bias=epst[0:1, :], scale=1.0 / D)
                rstd = kstat.tile([1, S], f32, tag="rstd")
                nc.vector.reciprocal(rstd[:], stdr[:])
                prstdb = kpsB.tile([128, S], f32, tag="big")
                for tch in range(4):
                    nc.tensor.matmul(
                        prstdb[:, tch * 512:(tch + 1) * 512], ones_p[:],
                        rstd[:, tch * 512:(tch + 1) * 512],
                        start=True, stop=True)
                # kn = (xc * gamma_d) * rstdB + beta_d
                kn = kstat.tile([128, S], f32, tag="kt_raw")
                nc.vector.scalar_tensor_tensor(
                    out=kn[:], in0=xc[:], scalar=gb[:], in1=prstdb[:],
                    op0=ALU.mult, op1=ALU.mult)
                nc.vector.tensor_scalar_add(kn[:], kn[:], bb_[:])
                # rope: kT[0:R] = kn[0:R]*cosT + (prot^T @ kn[0:R])*sinT
                prot_ps = kpsB.tile([R, S], f32, tag="big")
                for tch in range(4):
                    nc.tensor.matmul(
                        prot_ps[:, tch * 512:(tch + 1) * 512], prot[:],
                        kn[0:R, tch * 512:(tch + 1) * 512],
                        start=True, stop=True)
                tpe1 = kstat.tile([R, S], f32, tag="xc")
                nc.vector.tensor_mul(tpe1[:], kn[0:R, :], cosT_sb[:])
                tpe2 = kstat.tile([R, S], f32, tag="tpe2")
                nc.vector.tensor_mul(tpe2[:], prot_ps[:], sinT_sb[:])
                nc.vector.tensor_add(kT[0:R, :], tpe1[:], tpe2[:])
                nc.scalar.copy(kT[R:, :], kn[R:, :])

        # ------- phase W: w-proj -> |w|, sign stationaries ------------
        with tc.tile_pool(name="wphase", bufs=3) as wp, \
             tc.tile_pool(name="wkeep", bufs=1) as wk_, \
             tc.tile_pool(name="wpsum", bufs=1, space="PSUM") as wps, \
             tc.tile_pool(name="wtps", bufs=2, space="PSUM") as wtps:
            pw = [wps.tile([128, H], f32, tag=f"pw{i}", name=f"pw{i}")
                  for i in range(NST)]
            for c in range(NKC):
                hoc = wp.tile([128, SL], f32, tag="hoc")
                nc.sync.dma_start(hoc[:], hownT[c * 128:(c + 1) * 128, :])
                wwc = wp.tile([128, H], f32, tag="wwc")
                nc.sync.dma_start(wwc[:], w_w[c * 128:(c + 1) * 128, :])
                for st in range(NST):
                    nc.tensor.matmul(
                        pw[st][:], hoc[:, st * 128:(st + 1) * 128],
                        wwc[:], start=(c == 0), stop=(c == NKC - 1))
            sgnT = [wk_.tile([32, 128], f32, tag=f"sgnT{i}",
                             name=f"sgnTt{i}") for i in range(NST)]
            for st in range(NST):
                nc.scalar.activation(abswt[st][:], pw[st][:], ACTF.Abs,
                                     scale=QSCALE)
                sg = wk_.tile([128, H], f32, tag=f"sg{st}")
                nc.scalar.activation(sg[:], pw[st][:], ACTF.Sign)
                pt = wtps.tile([32, 128], f32, tag="pt")
                nc.tensor.transpose(pt[:], sg[:], ident[:])
                nc.scalar.copy(sgnT[st][:], pt[:])
            # sgnstat[32*r + h, 32*g + 4*(g%8) + r] = sgnT[st][h, q_loc]
            # with g = st*32 + 8*b + i, q_loc = 32*b + 4*i + r.
            nc.gpsimd.memset(sgnstat[:], 0.0)
            for st in range(NST):
                for r in range(4):
                    for b in range(4):
                        col = 1024 * st + 256 * b + r
                        anchor = sgnstat[32 * r:32 * r + 32, col:col + 1]
                        dst = bass.AP(tensor=anchor.tensor,
                                      offset=anchor.offset,
                                      ap=[anchor.ap[0], [36, 8]])
                        src = sgnT[st][:, 32 * b + r:32 * b + r + 29:4]
                        nc.sync.dma_start(dst, src)

        # ------- phase Q: q-proj + |w|-scale + rope + transpose -------
        for hd in range(8):
            pq = [qps.tile([128, 512], f32, tag=f"pq{i}", name=f"pq{i}")
                  for i in range(NST)]
            for c in range(NQC):
                wqc = qp.tile([128, 512], f32, tag="wqc")
                nc.sync.dma_start(
                    wqc[:], w_qb[c * 128:(c + 1) * 128,
                                 hd * 512:(hd + 1) * 512])
                for st in range(NST):
                    nc.tensor.matmul(
                        pq[st][:],
                        qr_t[:, c * SL + st * 128:c * SL + (st + 1) * 128],
                        wqc[:], start=(c == 0), stop=(c == NQC - 1))
            for st in range(NST):
                nc.vector.tensor_mul(
                    _hview(qsc[st][:, hd * 512:(hd + 1) * 512], 4, D),
                    _hview(pq[st][:], 4, D),
                    abswt[st][:, hd * 4:(hd + 1) * 4]
                    .unsqueeze(-1).to_broadcast([128, 4, D]))
        # rope (q2' first since q1' overwrites x1)
        for st in range(NST):
            cq = qtmp.tile([128, R], f32, tag="cq")
            nc.sync.dma_start(cq[:], cosq[st * 128:(st + 1) * 128, :])
            sq_ = qtmp.tile([128, R], f32, tag="sq_")
            nc.sync.dma_start(sq_[:], sinq[st * 128:(st + 1) * 128, :])
            v = _hview(qsc[st][:], H, D)
            x1, x2 = v[:, :, 0:HR], v[:, :, HR:R]
            c1 = _bch(cq[:, 0:HR], H, HR)
            c2 = _bch(cq[:, HR:R], H, HR)
            s1 = _bch(sq_[:, 0:HR], H, HR)
            s2 = _bch(sq_[:, HR:R], H, HR)
            ta = qtmp.tile([128, H * HR], f32, tag="ta")
            tb = qtmp.tile([128, H * HR], f32, tag="tb")
            td = qtmp.tile([128, H * HR], f32, tag="td")
            va, vb, vd = (_hview(t[:], H, HR) for t in (ta, tb, td))
            nc.vector.tensor_mul(va, x1, c1)      # x1*c1
            nc.vector.tensor_mul(vb, x2, s1)      # x2*s1
            nc.vector.tensor_mul(vd, x1, s2)      # x1*s2 (before x1 clobbered)
            nc.vector.tensor_mul(x1, x2, c2)      # x1 slot <- x2*c2
            nc.vector.tensor_add(x2, x1, vd)      # x2' = x2*c2 + x1*s2
            nc.vector.tensor_sub(x1, va, vb)      # x1' = x1*c1 - x2*s1
            # transpose per head into qT [d, (s, h)]: col = s_loc*H + h
            for h in range(H):
                ptq = qtps.tile([128, 128], f32, tag="ptq")
                nc.tensor.transpose(
                    ptq[:], qsc[st][:, h * D:(h + 1) * D], ident[:])
                base = st * 128 * H + h
                nc.scalar.copy(qT[:, base:base + 127 * H + 1:H], ptq[:])

        kwq_tps.__exit__(None, None, None)
        kwq_psum.__exit__(None, None, None)

        # ------- phase S: scores + h-contract + sort ------------------
        with tc.tile_pool(name="sphase", bufs=3) as sp, \
             tc.tile_pool(name="sortp", bufs=1) as sop, \
             tc.tile_pool(name="spsum", bufs=2, space="PSUM") as sps, \
             tc.tile_pool(name="ipsum", bufs=2, space="PSUM") as ips:
            vB_t = [sop.tile([128, T], f32, tag=f"vB{i}", name=f"vB{i}")
                    for i in range(NST)]
            iA_t = [sop.tile([128, T], i16, tag=f"iA{i}", name=f"iA{i}")
                    for i in range(NST)]
            iB_t = [sop.tile([128, T], i16, tag=f"iB{i}", name=f"iB{i}")
                    for i in range(NST)]
            msk_t = [sop.tile([128, T // 2], u16, tag=f"mk{i}",
                              name=f"mk{i}") for i in range(NST)]
            dt_t = [sop.tile([128, T // 2], i16, tag=f"dt{i}",
                             name=f"dt{i}") for i in range(NST)]
            md_t = [sop.tile([128, T // 2], i16, tag=f"md{i}",
                             name=f"md{i}") for i in range(NST)]

            def _substage(lo_v, hi_v, lo_vo, hi_vo,
                          lo_i, hi_i, lo_io, hi_io, mv, dv, mdv):
                nc.vector.tensor_tensor(out=mv, in0=lo_v, in1=hi_v,
                                        op=ALU.is_lt)
                nc.vector.tensor_tensor(out=lo_vo, in0=lo_v, in1=hi_v,
                                        op=ALU.max)
                nc.vector.tensor_tensor(out=hi_vo, in0=lo_v, in1=hi_v,
                                        op=ALU.min)
                nc.vector.tensor_sub(dv, hi_i, lo_i)
                nc.vector.tensor_mul(mdv, mv, dv)
                nc.vector.tensor_add(lo_io, lo_i, mdv)
                nc.vector.tensor_sub(hi_io, hi_i, mdv)

            def _sort_level(st, bs, cur, c0, cn):
                """One bitonic merge level (mirror + plain substages) on
                columns [c0, c0+cn) of s-tile st. Returns new ping-pong
                parity. Temps use column range [c0//2, (c0+cn)//2)."""
                vA, vB = is_sb[st], vB_t[st]
                iA, iB = iA_t[st], iB_t[st]
                bufs = [(vA, iA), (vB, iB)]
                m0 = c0 // 2
                mv0 = msk_t[st][:, m0:m0 + cn // 2]
                dv0 = dt_t[st][:, m0:m0 + cn // 2]
                md0 = md_t[st][:, m0:m0 + cn // 2]
                (va, ia), (vb, ib) = bufs[cur], bufs[1 - cur]

                def views(t, width):
                    return t[:, c0:c0 + cn].rearrange(
                        "p (nb x) -> p nb x", x=width)

                vsrc, vdst = views(va, 2 * bs), views(vb, 2 * bs)
                isrc, idst = views(ia, 2 * bs), views(ib, 2 * bs)
                mv = mv0.rearrange("p (nb x) -> p nb x", x=bs)
                dv = dv0.rearrange("p (nb x) -> p nb x", x=bs)
                mdv = md0.rearrange("p (nb x) -> p nb x", x=bs)
                _substage(
                    vsrc[:, :, 0:bs], vsrc[:, :, 2 * bs - 1:bs - 1:-1],
                    vdst[:, :, 0:bs], vdst[:, :, 2 * bs - 1:bs - 1:-1],
                    isrc[:, :, 0:bs], isrc[:, :, 2 * bs - 1:bs - 1:-1],
                    idst[:, :, 0:bs], idst[:, :, 2 * bs - 1:bs - 1:-1],
                    mv, dv, mdv)
                cur = 1 - cur
                j = bs // 2
                while j >= 1:
                    (va, ia), (vb, ib) = bufs[cur], bufs[1 - cur]

                    def views2(t):
                        return t[:, c0:c0 + cn].rearrange(
                            "p (nb two j) -> p nb two j", two=2, j=j)

                    vsrc, vdst = views2(va), views2(vb)
                    isrc, idst = views2(ia), views2(ib)
                    mv = mv0.rearrange("p (nb x) -> p nb x", x=j)
                    dv = dv0.rearrange("p (nb x) -> p nb x", x=j)
                    mdv = md0.rearrange("p (nb x) -> p nb x", x=j)
                    _substage(
                        vsrc[:, :, 0, :], vsrc[:, :, 1, :],
                        vdst[:, :, 0, :], vdst[:, :, 1, :],
                        isrc[:, :, 0, :], isrc[:, :, 1, :],
                        idst[:, :, 0, :], idst[:, :, 1, :],
                        mv, dv, mdv)
                    cur = 1 - cur
                    j //= 2
                return cur

            def emit_sort_half(st, th):
                """Sort columns [th*1024, (th+1)*1024) of is_sb[st]
                descending (bitonic levels bs=1..512). 55 substages ->
                ends with data in buffer B."""
                if th == 0:
                    nc.gpsimd.iota(iA_t[st][:], pattern=[[1, T]], base=0,
                                   channel_multiplier=0)
                if debug_scores:
                    nc.sync.dma_start(
                        sc_out[st * 128:(st + 1) * 128,
                               th * 1024:(th + 1) * 1024],
                        is_sb[st][:, th * 1024:(th + 1) * 1024])
                cur = 0
                bs = 1
                while bs < T // 2:
                    cur = _sort_level(st, bs, cur, th * 1024, 1024)
                    bs *= 2
                assert cur == 1

            def emit_sort_merge(st):
                """Final bitonic merge (bs=1024) over the full row; input
                in buffer B (parity 1), output lands back in A."""
                cur = _sort_level(st, T // 2, 1, 0, T)
                assert cur == 0
                ix32 = vB_t[st][:].bitcast(i32)
                nc.vector.tensor_copy(ix32, iA_t[st][:])
                nc.sync.dma_start(
                    idx_out[st * 128:(st + 1) * 128, :], ix32)

            for st in range(NST):
                for th in range(2):
                    ip = ips.tile([128, 1024], f32, tag="ip")
                    for g2 in range(32):
                        scp = sps.tile([128, 1024], f32, tag="scp")
                        cb = (st * 128 + 4 * g2) * H
                        lhs = qT[:, cb:cb + 128]
                        for ch in range(2):
                            nc.tensor.matmul(
                                scp[:, ch * 512:(ch + 1) * 512],
                                lhs,
                                kT[:, th * 1024 + ch * 512:
                                   th * 1024 + (ch + 1) * 512],
                                start=True, stop=True)
                        srl = sp.tile([128, 1024], f32, tag="srl")
                        nc.scalar.activation(srl[:], scp[:], ACTF.Relu)
                        b = g2 // 8
                        stat = sgnstat[:, 32 * (st * 32 + g2):
                                       32 * (st * 32 + g2) + 32]
                        for ch in range(2):
                            nc.tensor.matmul(
                                ip[32 * b:32 * b + 32,
                                   ch * 512:(ch + 1) * 512],
                                stat, srl[:, ch * 512:(ch + 1) * 512],
                                start=(g2 % 8 == 0), stop=(g2 % 8 == 7),
                                tile_position=(0, 32 * b))
                    nc.scalar.copy(
                        is_sb[st][:, th * 1024:(th + 1) * 1024], ip[:])
                    emit_sort_half(st, th)
                emit_sort_merge(st)

    _split_excess_waits(nc)
    return nc


_CACHE = {}


def _get_nc(debug_scores=False):
    key = ("nc", debug_scores)
    if key not in _CACHE:
        _CACHE[key] = _build(debug_scores)
    return _CACHE[key]


def _prep_inputs(hidden_states, q_resid, cos, sin, W_qb, W_k, ln_gamma,
                 ln_beta, W_w):
    f = np.float32
    hid = np.asarray(hidden_states, f)[0]            # [S, HID]
    qr = np.asarray(q_resid, f)[0]                   # [S, QL]
    cosx = np.ascontiguousarray(np.asarray(cos, f)[0])
    sinx = np.ascontiguousarray(np.asarray(sin, f)[0])
    hidT = np.ascontiguousarray(hid.T)               # [HID, S]
    qrT = np.ascontiguousarray(qr.T)                 # [QL, S]
    common = {
        "hidT": hidT,
        "cosT": np.ascontiguousarray(cosx.T),
        "sinT": np.ascontiguousarray(sinx.T),
        "w_qb": np.ascontiguousarray(np.asarray(W_qb, f)),
        "w_k": np.ascontiguousarray(np.asarray(W_k, f)),
        "w_w": np.ascontiguousarray(np.asarray(W_w, f)),
        "gammaC": np.ascontiguousarray(
            np.asarray(ln_gamma, f).reshape(D, 1)),
        "betaC": np.ascontiguousarray(np.asarray(ln_beta, f).reshape(D, 1)),
    }
    in_maps = []
    for c in range(NCORES):
        sl = slice(c * SL, (c + 1) * SL)
        m = dict(common)
        m["hownT"] = np.ascontiguousarray(hidT[:, sl])
        m["qrT"] = np.ascontiguousarray(qrT[:, sl])
        m["cosq"] = np.ascontiguousarray(cosx[sl])
        m["sinq"] = np.ascontiguousarray(sinx[sl])
        in_maps.append(m)
    return in_maps


last_result = None


def kernel(hidden_states, q_resid, cos, sin, W_qb, W_k, ln_gamma, ln_beta,
           W_w, _debug_scores=False, _trace=False):
    global last_result
    from concourse.bass_utils import run_bass_kernel_spmd
    nc = _get_nc(_debug_scores)
    in_maps = _prep_inputs(hidden_states, q_resid, cos, sin, W_qb, W_k,
                           ln_gamma, ln_beta, W_w)
    res = run_bass_kernel_spmd(nc, in_maps, list(range(NCORES)),
                               trace=_trace)
    last_result = res
    idx = np.concatenate([res.results[c]["idx_out"] for c in range(NCORES)],
                         axis=0).astype(np.int32)
    if _debug_scores:
        sc = np.concatenate([res.results[c]["sc_out"]
                             for c in range(NCORES)], axis=0)
        return idx[None], sc[None]
    return idx[None]

